# revision 9
# baseline (speedup 1.0000x reference)
"""Batched complex linear solve  A x = b  (A = A_r + i*A_i, b = b_r + i*b_i).

Shapes: A [8192, 64, 64], b [8192, 64, 16], fp32 real/imag planes.
Returns (real(x), imag(x)) float32, matching the reference.

Everything runs on the 8 trn2 NeuronCores (1024 systems per core, pure batch
parallelism).  Per system, the complex matrix is embedded as the real
128x128 block matrix M = [[Ar, -Ai], [Ai, Ar]].  The device computes the
SPD Gram matrix P = M^T M with one matmul, inverts it with a scaled
Newton-Schulz iteration (all iterates are symmetric polynomials in P, so
every matmul can use the stored tile directly as the pre-transposed
stationary operand - no transposes in the hot loop), then solves
x = P^{-1} M^T b with three M-space iterative-refinement steps.  The
refinement residual r = b - Mx is shipped back so the host can re-solve the
handful of nearly-singular systems (lambda_min(P) below ~1e-3) with LAPACK.

The per-iteration interval-balancing scales (computed from the global
eigenvalue range [LAM_LO, LAM_HI] of P) double the Newton-Schulz
convergence rate in the early regime: the lower edge of the normalized
eigenvalue interval grows ~4x per iteration instead of ~2x.
"""

import math
import os
import time

import numpy as np

B, N, K = 8192, 64, 16
NCORES = 8
NSYS = B // NCORES   # systems per core
G = 8                # systems per tile group
NGRP = 2             # groups interleaved per loop body
BODY = G * NGRP      # systems per hardware-loop body
K_NS = 12            # scaled Newton-Schulz iterations (first one is matmul-free)
N_REF = 3            # iterative-refinement steps
LAM_LO, LAM_HI = 1e-3, 800.0   # eigenvalue range of P the schedule covers
FLAG_TOL = 3e-3      # host re-solve when ||b - M x||/||b|| exceeds this

LAST_EXEC_NS = None


def _ns_schedule():
    """Interval-balancing pre-scales s_j for u' = f(s*u), f(u) = u(2-u)."""
    m = LAM_LO / LAM_HI
    scales = []
    for _ in range(K_NS):
        scales.append(2.0 / (1.0 + m))
        m = 4.0 * m / ((1.0 + m) ** 2)
    return scales, m


def _split_excess_waits(nc, mybir, max_waits=1):
    # This toolchain's walrus accepts at most one semaphore wait per
    # instruction; move excess waits onto same-engine nops inserted before
    # the offending instruction.
    for bbname, bbobj in list(nc.bb_map.items()):
        raw = bbobj.bb
        insts = list(raw.instructions)
        out, changed = [], False
        for inst in insts:
            si = getattr(inst, "sync_info", None)
            waits = list(si.on_wait) if si and si.on_wait else []
            if len(waits) > max_waits:
                eng = inst.engine
                excess, keep = waits[:-max_waits], waits[-max_waits:]
                for w in excess:
                    bi = nc.engines[eng].nop(nofuse=True)
                    nop_inst = bi.ins
                    for bb2 in nc.bb_map.values():
                        lst = list(bb2.bb.instructions)
                        if lst and lst[-1].name == nop_inst.name:
                            bb2.bb.instructions = lst[:-1]
                            break
                    nsi = nop_inst.sync_info
                    if nsi is None:
                        nop_inst.sync_info = mybir.SyncInfo(
                            on_wait=[w], on_update=[]
                        )
                    else:
                        nsi.on_wait = [w]
                    out.append(nop_inst)
                si.on_wait = keep
                changed = True
            out.append(inst)
        if changed:
            raw.instructions = out


def _build_nc(nsys=NSYS):
    import concourse.bass as bass
    import concourse.tile as tile
    from concourse import mybir
    from concourse.bass import ds
    from concourse.masks import make_identity

    F32 = mybir.dt.float32
    MULT = mybir.AluOpType.mult
    ADD = mybir.AluOpType.add
    COPY = mybir.ActivationFunctionType.Copy

    scales, m_final = _ns_schedule()
    a0 = scales[0] / LAM_HI           # W0 = a0 * I (pre-scaled)
    c1 = scales[1] * a0               # W1 = c1 * (2I - a0*P)

    nbody = nsys // BODY

    nc = bass.Bass()
    Ar_d = nc.declare_dram_parameter("Ar", [nsys, N, N], F32, isOutput=False)
    Ai_d = nc.declare_dram_parameter("Ai", [nsys, N, N], F32, isOutput=False)
    br_d = nc.declare_dram_parameter("br", [nsys, N, K], F32, isOutput=False)
    bi_d = nc.declare_dram_parameter("bi", [nsys, N, K], F32, isOutput=False)
    xh_d = nc.declare_dram_parameter("xh", [2 * N, nsys, K], F32, isOutput=True)
    rh_d = nc.declare_dram_parameter("rh", [2 * N, nsys, K], F32, isOutput=True)

    with tile.TileContext(nc) as tc:
        with (
            tc.tile_pool(name="const_p", bufs=1) as const_p,
            tc.tile_pool(name="m_p", bufs=1) as m_p,
            tc.tile_pool(name="big_p", bufs=1) as big_p,
            tc.tile_pool(name="small_p", bufs=1) as small_p,
            tc.tile_pool(name="big_ps", bufs=1, space="PSUM") as big_ps,
            tc.tile_pool(name="sol_ps0", bufs=2, space="PSUM") as sol_ps0,
            tc.tile_pool(name="sol_ps1", bufs=2, space="PSUM") as sol_ps1,
        ):
            ident = const_p.tile([128, 128], F32)
            make_identity(nc, ident[:])
            twoIG = const_p.tile([128, G, 128], F32)
            nc.gpsimd.memset(twoIG[:], 0.0)
            for s in range(G):
                nc.gpsimd.affine_select(
                    out=twoIG[:, s, :],
                    in_=twoIG[:, s, :],
                    compare_op=mybir.AluOpType.not_equal,
                    fill=2.0,
                    base=0,
                    pattern=[[-1, 128]],
                    channel_multiplier=1,
                )

            # per-group persistent tiles
            Mt = [m_p.tile([128, G, 128], F32, name=f"Mt{g}") for g in range(NGRP)]
            MTt = [m_p.tile([128, G, 128], F32, name=f"MTt{g}") for g in range(NGRP)]
            Psb = [big_p.tile([128, G, 128], F32, name=f"Psb{g}") for g in range(NGRP)]
            Wsb = [big_p.tile([128, G, 128], F32, name=f"Wsb{g}") for g in range(NGRP)]
            Rsb = [big_p.tile([128, G, 128], F32, name=f"Rsb{g}") for g in range(NGRP)]
            bemb = [small_p.tile([128, G, K], F32, name=f"bemb{g}") for g in range(NGRP)]
            csb = [small_p.tile([128, G, K], F32, name=f"csb{g}") for g in range(NGRP)]
            xsb = [small_p.tile([128, G, K], F32, name=f"xsb{g}") for g in range(NGRP)]
            rsb = [small_p.tile([128, G, K], F32, name=f"rsb{g}") for g in range(NGRP)]
            rhosb = [small_p.tile([128, G, K], F32, name=f"rhosb{g}") for g in range(NGRP)]

            with tc.For_i(0, nsys, BODY) as g0:
                offs = [g0 if g == 0 else g0 + g * G for g in range(NGRP)]

                # ---- load A quadrants and b halves
                for g in range(NGRP):
                    sl = ds(offs[g], G)
                    nc.sync.dma_start(
                        Mt[g][0:64, :, 0:64], Ar_d[sl].rearrange("g r c -> r g c")
                    )
                    nc.sync.dma_start(
                        Mt[g][64:128, :, 64:128], Ar_d[sl].rearrange("g r c -> r g c")
                    )
                    nc.sync.dma_start(
                        Mt[g][64:128, :, 0:64], Ai_d[sl].rearrange("g r c -> r g c")
                    )
                    nc.sync.dma_start(
                        Mt[g][0:64, :, 64:128], Ai_d[sl].rearrange("g r c -> r g c")
                    )
                    nc.sync.dma_start(
                        bemb[g][0:64, :, :], br_d[sl].rearrange("g r k -> r g k")
                    )
                    nc.sync.dma_start(
                        bemb[g][64:128, :, :], bi_d[sl].rearrange("g r k -> r g k")
                    )
                # negate the top-right quadrant in place: M = [[Ar,-Ai],[Ai,Ar]]
                for g in range(NGRP):
                    nc.vector.tensor_scalar_mul(
                        Mt[g][0:64, :, 64:128], Mt[g][0:64, :, 64:128], -1.0
                    )

                # ---- build M^T via PE transposes (left half -> top rows,
                # right half -> bottom rows, landing on the right partitions);
                # the transposes stage through the big PSUM tiles, which the
                # P matmuls overwrite afterwards.
                Tps = [
                    big_ps.tile([128, G, 128], F32, name=f"Tps{g}")
                    for g in range(NGRP)
                ]
                for g in range(NGRP):
                    for s in range(G):
                        nc.tensor.transpose(
                            Tps[g][0:64, s, :], Mt[g][:, s, 0:64], ident[:]
                        )
                        nc.tensor.transpose(
                            Tps[g][64:128, s, :], Mt[g][:, s, 64:128], ident[:]
                        )
                for g in range(NGRP):
                    nc.scalar.copy(MTt[g][:], Tps[g][:])

                # ---- P = M^T M
                for g in range(NGRP):
                    for s in range(G):
                        nc.tensor.matmul(
                            Tps[g][:, s, :], Mt[g][:, s, :], Mt[g][:, s, :],
                            start=True, stop=True,
                        )
                for g in range(NGRP):
                    nc.scalar.copy(Psb[g][:], Tps[g][:])

                # ---- NS iteration 1 (matmul-free): W = c1*(2I - a0*P)
                for g in range(NGRP):
                    nc.vector.scalar_tensor_tensor(
                        out=Rsb[g][:], in0=Psb[g][:], scalar=-a0,
                        in1=twoIG[:], op0=MULT, op1=ADD,
                    )
                for g in range(NGRP):
                    nc.scalar.mul(Wsb[g][:], Rsb[g][:], c1)

                # ---- NS iterations 2..K_NS.  The matmul computes
                # V = W^T (2I - P W); that form amplifies any asymmetry of W
                # by ~2*sigma per iteration, so every other iteration we
                # re-symmetrize W = (V + V^T)/2 via a PE transpose (the
                # transpose stages through the just-freed PSUM tile).
                for j in range(1, K_NS):
                    sigma = scales[j + 1] if j + 1 < K_NS else 1.0
                    do_sym = (j % 2) == 1
                    for g in range(NGRP):
                        for s in range(G):
                            nc.tensor.matmul(
                                Tps[g][:, s, :], Psb[g][:, s, :], Wsb[g][:, s, :],
                                start=True, stop=True,
                            )
                    for g in range(NGRP):
                        nc.vector.scalar_tensor_tensor(
                            out=Rsb[g][:], in0=Tps[g][:], scalar=-1.0,
                            in1=twoIG[:], op0=MULT, op1=ADD,
                        )
                    for g in range(NGRP):
                        for s in range(G):
                            nc.tensor.matmul(
                                Tps[g][:, s, :], Wsb[g][:, s, :], Rsb[g][:, s, :],
                                start=True, stop=True,
                            )
                    if do_sym:
                        # Rsb <- (sigma/2) V, then Tps <- Rsb^T, then
                        # Wsb <- Rsb + Tps = sigma*(V + V^T)/2
                        for g in range(NGRP):
                            nc.scalar.mul(Rsb[g][:], Tps[g][:], sigma / 2.0)
                        for g in range(NGRP):
                            for s in range(G):
                                nc.tensor.transpose(
                                    Tps[g][:, s, :], Rsb[g][:, s, :], ident[:]
                                )
                        for g in range(NGRP):
                            nc.vector.tensor_tensor(
                                out=Wsb[g][:], in0=Rsb[g][:], in1=Tps[g][:], op=ADD
                            )
                    else:
                        for g in range(NGRP):
                            if sigma == 1.0:
                                nc.scalar.copy(Wsb[g][:], Tps[g][:])
                            else:
                                nc.scalar.mul(Wsb[g][:], Tps[g][:], sigma)

                # ---- solve x = W * (M^T b) with M-space refinement
                sol_ps = [sol_ps0, sol_ps1]

                def sp(g):
                    return sol_ps[g].tile([128, G, K], F32, name="sp")

                cps = []
                for g in range(NGRP):
                    cp = sp(g)
                    cps.append(cp)
                    for s in range(G):
                        nc.tensor.matmul(
                            cp[:, s, :], Mt[g][:, s, :], bemb[g][:, s, :],
                            start=True, stop=True,
                        )
                for g in range(NGRP):
                    nc.scalar.copy(csb[g][:], cps[g][:])
                xps = []
                for g in range(NGRP):
                    xp = sp(g)
                    xps.append(xp)
                    for s in range(G):
                        nc.tensor.matmul(
                            xp[:, s, :], Wsb[g][:, s, :], csb[g][:, s, :],
                            start=True, stop=True,
                        )
                for g in range(NGRP):
                    nc.scalar.copy(xsb[g][:], xps[g][:])

                for ref in range(N_REF):
                    mxps = []
                    for g in range(NGRP):
                        mp = sp(g)
                        mxps.append(mp)
                        for s in range(G):
                            nc.tensor.matmul(
                                mp[:, s, :], MTt[g][:, s, :], xsb[g][:, s, :],
                                start=True, stop=True,
                            )
                    for g in range(NGRP):
                        nc.vector.scalar_tensor_tensor(
                            out=rsb[g][:], in0=mxps[g][:], scalar=-1.0,
                            in1=bemb[g][:], op0=MULT, op1=ADD,
                        )
                    if ref == N_REF - 1:
                        for g in range(NGRP):
                            nc.sync.dma_start(
                                rh_d[:, ds(offs[g], G), :], rsb[g][:]
                            )
                    rps = []
                    for g in range(NGRP):
                        rp = sp(g)
                        rps.append(rp)
                        for s in range(G):
                            nc.tensor.matmul(
                                rp[:, s, :], Mt[g][:, s, :], rsb[g][:, s, :],
                                start=True, stop=True,
                            )
                    for g in range(NGRP):
                        nc.scalar.copy(rhosb[g][:], rps[g][:])
                    dxps = []
                    for g in range(NGRP):
                        dp = sp(g)
                        dxps.append(dp)
                        for s in range(G):
                            nc.tensor.matmul(
                                dp[:, s, :], Wsb[g][:, s, :], rhosb[g][:, s, :],
                                start=True, stop=True,
                            )
                    for g in range(NGRP):
                        nc.vector.tensor_tensor(
                            out=xsb[g][:], in0=xsb[g][:], in1=dxps[g][:], op=ADD
                        )

                for g in range(NGRP):
                    nc.sync.dma_start(xh_d[:, ds(offs[g], G), :], xsb[g][:])

    _split_excess_waits(nc, mybir)
    return nc


def _host_algorithm(Ar, Ai, br, bi):
    """Numpy replica of the device algorithm (fallback + testing)."""
    f32 = np.float32
    scales, _ = _ns_schedule()
    a0 = scales[0] / LAM_HI
    c1 = scales[1] * a0
    nb = Ar.shape[0]
    M = np.zeros((nb, 128, 128), f32)
    M[:, :64, :64] = Ar; M[:, :64, 64:] = -Ai
    M[:, 64:, :64] = Ai; M[:, 64:, 64:] = Ar
    MT = np.swapaxes(M, 1, 2)
    bemb = np.concatenate([br, bi], axis=1).astype(f32)
    P = (MT @ M).astype(f32)
    Iden = np.eye(128, dtype=f32)
    W = (c1 * (2 * Iden[None] - a0 * P)).astype(f32)
    for j in range(1, K_NS):
        sigma = scales[j + 1] if j + 1 < K_NS else 1.0
        T = (P @ W).astype(f32)
        R = (2 * Iden[None] - T).astype(f32)
        W = (sigma * (W @ R)).astype(f32)
    c = (MT @ bemb).astype(f32)
    x = (W @ c).astype(f32)
    r_out = None
    for ref in range(N_REF):
        r = (bemb - (M @ x).astype(f32)).astype(f32)
        if ref == N_REF - 1:
            r_out = r
        x = (x + (W @ (MT @ r).astype(f32)).astype(f32)).astype(f32)
    return x, r_out


def _device_solve(Ar, Ai, br, bi, n_timing_runs=3):
    """Run the Bass kernel on the 8 cores via the axon PJRT backend.

    Mirrors bass2jax.run_bass_via_pjrt's shard_map path, but jits once and
    keeps inputs device-resident so repeated executions measure the actual
    dispatch+execute time (no NTFF profiling exists under this axon build).
    LAST_EXEC_NS is the minimum wall time over the timing runs.
    """
    global LAST_EXEC_NS
    import jax
    import jax.numpy as jnp
    from jax.experimental.shard_map import shard_map
    from jax.sharding import Mesh, PartitionSpec
    from concourse import bass2jax, mybir

    devices = jax.devices()[:NCORES]
    if len(devices) < NCORES or devices[0].platform == "cpu":
        raise RuntimeError("axon neuron devices unavailable")

    bass2jax.install_neuronx_cc_hook()
    nc = _build_nc(NSYS)

    in_names, in_meta, out_names, out_avals, zero_outs = [], {}, [], [], []
    for alloc in nc.m.functions[0].allocations:
        if not isinstance(alloc, mybir.MemoryLocationSet):
            continue
        name = alloc.memorylocations[0].name
        if alloc.kind == "ExternalInput":
            in_names.append(name)
            in_meta[name] = (tuple(alloc.tensor_shape), mybir.dt.np(alloc.dtype))
        elif alloc.kind == "ExternalOutput":
            out_names.append(name)
            shape = tuple(alloc.tensor_shape)
            dtype = mybir.dt.np(alloc.dtype)
            out_avals.append(jax.core.ShapedArray(shape, dtype))
            zero_outs.append(np.zeros(shape, dtype))
    n_params = len(in_names)
    all_names = in_names + out_names

    def _body(*args):
        outs = bass2jax._bass_exec_p.bind(
            *args,
            out_avals=tuple(out_avals),
            in_names=tuple(all_names),
            out_names=tuple(out_names),
            lowering_input_output_aliases=(),
            sim_require_finite=True,
            sim_require_nnan=True,
            nc=nc,
        )
        return tuple(outs)

    mesh = Mesh(np.asarray(devices), ("core",))
    nin = n_params + len(out_names)
    sharded = jax.jit(
        shard_map(
            _body, mesh=mesh,
            in_specs=(PartitionSpec("core"),) * nin,
            out_specs=(PartitionSpec("core"),) * len(out_names),
            check_rep=False,
        ),
        keep_unused=True,
    )

    from jax.sharding import NamedSharding

    shd = NamedSharding(mesh, PartitionSpec("core"))
    host_in = {"Ar": Ar, "Ai": Ai, "br": br, "bi": bi}

    def _full_input(n):
        if n in host_in:
            return host_in[n]
        shape, dt = in_meta[n]   # e.g. dbg_addr: replicate zeros per core
        return np.zeros((NCORES * shape[0],) + tuple(shape[1:]), dt)

    dev_args = [jax.device_put(_full_input(n), shd) for n in in_names]
    dev_args += [
        jax.device_put(
            np.zeros((NCORES * z.shape[0],) + z.shape[1:], z.dtype), shd
        )
        for z in zero_outs
    ]
    outs = sharded(*dev_args)      # compile + first run
    jax.block_until_ready(outs)
    best = None
    for _ in range(max(n_timing_runs, 1)):
        t0 = time.perf_counter()
        outs = sharded(*dev_args)
        jax.block_until_ready(outs)
        dt = time.perf_counter() - t0
        best = dt if best is None else min(best, dt)
    LAST_EXEC_NS = int(best * 1e9)

    res = {n: np.asarray(v) for n, v in zip(out_names, outs)}
    # outputs come back concatenated over cores along axis 0: [8*128, NSYS, K]
    xh = res["xh"].reshape(NCORES, 2 * N, NSYS, K)
    rh = res["rh"].reshape(NCORES, 2 * N, NSYS, K)
    x = np.ascontiguousarray(
        np.concatenate(list(xh), axis=1).transpose(1, 0, 2)
    )  # [B, 128, K]
    r = np.ascontiguousarray(np.concatenate(list(rh), axis=1).transpose(1, 0, 2))
    return x, r


def kernel(tensor_A_r, tensor_A_i, tensor_b_r, tensor_b_i):
    Ar = np.ascontiguousarray(tensor_A_r, np.float32)
    Ai = np.ascontiguousarray(tensor_A_i, np.float32)
    br = np.ascontiguousarray(tensor_b_r, np.float32)
    bi = np.ascontiguousarray(tensor_b_i, np.float32)

    try:
        x, r = _device_solve(Ar, Ai, br, bi)
    except Exception:
        x, r = _host_algorithm(Ar, Ai, br, bi)

    # flag systems whose M-space residual is too large and re-solve on host
    bnorm2 = (br.astype(np.float64) ** 2).sum(axis=(1, 2)) + \
             (bi.astype(np.float64) ** 2).sum(axis=(1, 2))
    rnorm2 = (r.astype(np.float64) ** 2).sum(axis=(1, 2))
    rel = np.sqrt(rnorm2 / np.maximum(bnorm2, 1e-300))
    bad = ~(rel < FLAG_TOL) | ~np.isfinite(rel)
    if bad.any():
        idx = np.nonzero(bad)[0]
        Ab = (Ar[idx] + 1j * Ai[idx]).astype(np.complex64)
        bb = (br[idx] + 1j * bi[idx]).astype(np.complex64)
        xb = np.linalg.solve(Ab, bb)
        x[idx, :64, :] = np.real(xb)
        x[idx, 64:, :] = np.imag(xb)

    xr = np.ascontiguousarray(x[:, :64, :], np.float32)
    xi = np.ascontiguousarray(x[:, 64:, :], np.float32)
    return (xr, xi)
